# revision 1
# baseline (speedup 1.0000x reference)
"""AdaptiveBlockSparseAttnTrain Trainium2 kernel (8 NeuronCores, head-parallel).

Per core (= one head), fused single pass over query-block groups:
  - Gilbert rearrange/unrearrange, padding, transposes, final division done
    host-side (cheap numpy); device computes the attention pipeline.
  - ST_ij = K_j @ Q_group^T on TensorE in fp16 (same precision class as tf32;
    measured zero energy-mask flips vs the f32 reference).
  - E = exp(ST * scale) on ScalarE -> fp16.
  - W[j, q] = sum_r E_ij[r, q] via 31 accumulating basis matmuls per group
    (the basis constant also encodes key validity for the partial last block).
  - den[q] = ones^T W, pooling row P_i[j] = sum_q W[j,q]/den[q] via tiny
    matmuls + a PE transpose of W's i-slice.
  - Energy mask, rank-based (equals reference argsort/cumsum/clip for
    tie-free inputs):  keep = (cum_incl < 0.95*tot & rank < 21) | rank < 1.
  - Mask chain batched across the group's 4 query blocks (32-partition
    stacking via tile_position) with fused scalar_tensor_tensor ops.
  - Mask multiply in place on E (one DVE pass per query block).
  - PV transposed and group-batched: O^T_group[d, q] += V_j^T @ E_masked_j
    (31 matmuls of N=512 per group; V stationary).
  - Masked denominator column = (W^T . m) via one fused DVE op; the final
    division by it and the [d, q] -> [q, d] transpose happen on the host.
"""

import sys
import types

sys.path.insert(0, "/opt/trn_rl_repo")

import numpy as np

# The NTFF profile path (fired when BASS_TRACE is set in the environment)
# imports antenv.axon_hooks, which this image does not ship. Register a stub
# so run_bass_kernel_spmd degrades gracefully (skips tracing) instead of
# crashing, without shadowing a real module if one exists.
try:
    import antenv.axon_hooks  # noqa: F401
except ImportError:
    _m = types.ModuleType("antenv.axon_hooks")
    _hook = {}
    _m.set_axon_ntff_profile_hook = lambda h: _hook.__setitem__("h", h)
    _m.get_axon_ntff_profile_hook = lambda: _hook.get("h")
    sys.modules["antenv.axon_hooks"] = _m

import concourse.bass as bass
import concourse.bacc as bacc
import concourse.tile as tile
from concourse import mybir
from concourse.bass_utils import run_bass_kernel_spmd

TEXT = 224
VID = 3696
SEQ = 3920
BLOCK = 128
NB = 31
SP = 3968
D = 128
NCORES = 8
NVLAST = SEQ - 30 * 128        # 80 valid tokens in the last block
SCALE = 1.0 / np.sqrt(128.0)

F32 = mybir.dt.float32
F16 = mybir.dt.float16

GROUPS = [(0, 4), (4, 4), (8, 4), (12, 4), (16, 4), (20, 4), (24, 4), (28, 3)]


def _bcast_ap(t_ap, mid_count, inner_count, inner_step, mid_step):
    """3-D broadcast AP [[part], [mid], [inner]] from a 2-D tile AP."""
    return bass.AP(
        tensor=t_ap.tensor,
        offset=t_ap.offset,
        ap=[list(t_ap.ap[0]), [mid_step, mid_count], [inner_step, inner_count]],
    )


def build_graph():
    nc = bacc.Bacc("TRN2", target_bir_lowering=False, debug=False,
                   num_devices=NCORES)
    qT_d = nc.dram_tensor("qT", [128, SP], F16, kind="ExternalInput").ap()
    kT_d = nc.dram_tensor("kT", [128, SP], F16, kind="ExternalInput").ap()
    vv_d = nc.dram_tensor("vv", [128, NB * 128], F16, kind="ExternalInput").ap()
    bas_d = nc.dram_tensor("bas", [128, NB * NB], F16, kind="ExternalInput").ap()
    i31_d = nc.dram_tensor("i31", [31, 31], F32, kind="ExternalInput").ap()
    i128_d = nc.dram_tensor("i128", [128, 128], F32, kind="ExternalInput").ap()
    outT_d = nc.dram_tensor("outT", [128, SP], F32, kind="ExternalOutput").ap()
    den_d = nc.dram_tensor("den", [128, NB], F32, kind="ExternalOutput").ap()

    with tile.TileContext(nc) as tc:
        with (
            tc.tile_pool(name="singles", bufs=1) as singles,
            tc.tile_pool(name="eg", bufs=3) as egp,
            tc.tile_pool(name="gw", bufs=2) as gwp,
            tc.tile_pool(name="small", bufs=8) as small,
            tc.tile_pool(name="outs", bufs=2) as outsp,
            tc.tile_pool(name="stps", bufs=3, space="PSUM") as stps,
            tc.tile_pool(name="wps", bufs=1, space="PSUM") as wps,
            tc.tile_pool(name="pvps", bufs=1, space="PSUM") as pvps,
            tc.tile_pool(name="mini", bufs=2, space="PSUM") as minips,
            tc.tile_pool(name="minB", bufs=1, space="PSUM") as minBps,
        ):
            # ---- resident inputs ----
            sq = singles.tile([128, SP], F16)
            sk = singles.tile([128, SP], F16)
            sv = singles.tile([128, NB, 128], F16)
            sbas = singles.tile([128, NB, NB], F16)
            i31 = singles.tile([31, 31], F32)
            i128 = singles.tile([128, 128], F32)
            nc.sync.dma_start(i128[:, :], i128_d)
            nc.sync.dma_start(sq[:, :], qT_d)
            nc.sync.dma_start(sk[:, :], kT_d)
            nc.sync.dma_start(sv[:, :, :], vv_d.rearrange("p (j w) -> p j w", j=NB))
            nc.sync.dma_start(sbas[:, :, :], bas_d.rearrange("p (j m) -> p j m", j=NB))
            nc.sync.dma_start(i31[:, :], i31_d)

            # ---- constants ----
            ones31_col = singles.tile([31, 1], F32)
            nc.vector.memset(ones31_col[:, :], 1.0)
            ones_row128 = singles.tile([1, 128], F32)
            nc.vector.memset(ones_row128[:, :], 1.0)
            ones_row31 = singles.tile([1, 31], F32)
            nc.vector.memset(ones_row31[:, :], 1.0)
            onesb31 = singles.tile([128, 31], F32)
            nc.vector.memset(onesb31[:, :], 1.0)
            den_sb = singles.tile([128, NB], F32)
            nc.vector.memset(den_sb[:, :], 1.0)

            egs = {}
            wsbs = {}
            chains = {}

            wpss = {}

            def emit_st(g, jlist):
                i0, G = GROUPS[g]
                GW = G * 128
                if g not in egs:
                    egs[g] = egp.tile([128, NB, GW], F16, tag="eg", name=f"eg{g}")
                    wpss[g] = wps.tile([31, GW], F32, tag="wps",
                                       name=f"wps{g}")
                eg = egs[g]
                w_ps = wpss[g]
                for j in jlist:
                    st = stps.tile([128, 512], F32, tag="st")
                    nc.tensor.matmul(
                        st[:, :GW],
                        sk[:, j * 128:(j + 1) * 128],
                        sq[:, i0 * 128:i0 * 128 + GW],
                        start=True, stop=True,
                    )
                    nc.scalar.activation(
                        eg[:, j, :], st[:, :GW],
                        mybir.ActivationFunctionType.Exp,
                        bias=0.0, scale=float(SCALE),
                    )
                    nc.tensor.matmul(
                        w_ps[:, :],
                        sbas[:, j, :],
                        eg[:, j, :],
                        start=(j == 0), stop=(j == NB - 1),
                    )

            def emit_w(g):
                w_sb = gwp.tile([31, GW], F32, tag="wsb") if False else gwp.tile([31, GROUPS[g][1]*128], F32, tag="wsb", name=f"wsb{g}")
                nc.vector.tensor_copy(w_sb[:, :], wpss[g][:, :])
                wsbs[g] = w_sb
                del wpss[g]

            def emit_chain_group(g):
                i0, G = GROUPS[g]
                eg = egs[g]
                w_sb = wsbs[g]

                # per-i: den column, 1/den, W^T slice, pooling col (stacked)
                pcol4_ps = minBps.tile([128, 1], F32, tag="minB")
                nc.vector.memset(pcol4_ps[:, :], 0.0)
                wts = []
                dcols = []
                rdws = []
                for il in range(G):
                    i = i0 + il
                    nv = NVLAST if i == 30 else 128
                    qs0 = il * 128
                    dcol_ps = minips.tile([128, 1], F32, tag="mini")
                    nc.tensor.matmul(dcol_ps[:nv, :], w_sb[:, qs0:qs0 + nv],
                                     ones31_col[:, :], start=True, stop=True)
                    dcols.append(dcol_ps)
                for il in range(G):
                    i = i0 + il
                    nv = NVLAST if i == 30 else 128
                    rdw = small.tile([128, 1], F32, tag="rdw", name=f"rdw{g}_{il}")
                    nc.vector.reciprocal(rdw[:nv, :], dcols[il][:nv, :])
                    rdws.append(rdw)
                for il in range(G):
                    i = i0 + il
                    nv = NVLAST if i == 30 else 128
                    qs0 = il * 128
                    wt_ps = minips.tile([128, 31], F32, tag="mini")
                    nc.tensor.matmul(wt_ps[:nv, :], w_sb[:, qs0:qs0 + nv],
                                     i31[:, :], start=True, stop=True)
                    wt_sb = small.tile([128, 31], F32, tag="wtsb",
                                       name=f"wt{g}_{il}")
                    nc.vector.tensor_copy(wt_sb[:nv, :], wt_ps[:nv, :])
                    wts.append((wt_sb, nv))
                for il in range(G):
                    i = i0 + il
                    nv = NVLAST if i == 30 else 128
                    nc.tensor.matmul(pcol4_ps[32 * il:32 * il + 31, :],
                                     wts[il][0][:nv, :], rdws[il][:nv, :],
                                     start=True, stop=True,
                                     tile_position=(0, 32 * il))
                pcol4 = small.tile([128, 1], F32, tag="pcol4")
                nc.vector.tensor_copy(pcol4[:, :], pcol4_ps[:, :])

                # all four pooling rows concatenated: [1, 128] = pcol4^T
                prow_ps = minips.tile([1, 128], F32, tag="mini")
                nc.tensor.matmul(prow_ps[:, :], pcol4[:, :], i128[:, :],
                                 start=True, stop=True)
                prow = small.tile([1, 128], F32, tag="prow")
                nc.vector.tensor_copy(prow[:, :], prow_ps[:, :])

                # Pb4: rows 32*il..32*il+30 = pooling row of query block i0+il
                pb_ps = minBps.tile([128, 31], F32, tag="minB")
                nc.vector.memset(pb_ps[:, :], 0.0)
                for il in range(G):
                    nc.tensor.matmul(pb_ps[32 * il:32 * il + 31, :],
                                     ones_row31[:, :],
                                     prow[:, 32 * il:32 * il + 31],
                                     start=True, stop=True,
                                     tile_position=(0, 32 * il))
                pb = small.tile([128, 31], F32, tag="pb")
                nc.vector.tensor_copy(pb[:, :], pb_ps[:, :])
                Gt = small.tile([128, 31], F32, tag="Gt")
                rank = small.tile([128, 1], F32, tag="rank")
                nc.vector.scalar_tensor_tensor(
                    Gt[:, :], pb[:, :], pcol4[:, :], onesb31[:, :],
                    mybir.AluOpType.is_gt, mybir.AluOpType.mult,
                    accum_out=rank[:, :])
                esum = small.tile([128, 1], F32, tag="esum")
                tmp = small.tile([128, 31], F32, tag="tmp")
                nc.vector.scalar_tensor_tensor(
                    tmp[:, :], pb[:, :], pcol4[:, :], pb[:, :],
                    mybir.AluOpType.is_ge, mybir.AluOpType.mult,
                    accum_out=esum[:, :])
                tot = small.tile([128, 1], F32, tag="tot")
                nc.vector.reduce_sum(tot[:, :], pb[:, :],
                                     axis=mybir.AxisListType.X)
                C = small.tile([128, 1], F32, tag="C")
                nc.vector.scalar_tensor_tensor(
                    C[:, :], tot[:, :], 0.95, esum[:, :],
                    mybir.AluOpType.mult, mybir.AluOpType.is_gt,
                )
                ca = small.tile([128, 1], F32, tag="ca")
                nc.vector.scalar_tensor_tensor(
                    ca[:, :], rank[:, :], 21.0, C[:, :],
                    mybir.AluOpType.is_lt, mybir.AluOpType.logical_and,
                )
                mv4 = small.tile([128, 1], F32, tag="mv4")
                nc.vector.scalar_tensor_tensor(
                    mv4[:, :], rank[:, :], 1.0, ca[:, :],
                    mybir.AluOpType.is_lt, mybir.AluOpType.logical_or,
                )

                # all four mask rows concatenated: [1, 128] = mv4^T
                mrow_ps = minips.tile([1, 128], F32, tag="mini")
                nc.tensor.matmul(mrow_ps[:, :], mv4[:, :], i128[:, :],
                                 start=True, stop=True)
                mrow = small.tile([1, 128], F32, tag="mrow")
                nc.vector.tensor_copy(mrow[:, :], mrow_ps[:, :])
                # mbc4[p, 32*il + j] = mask_{i0+il}[j]
                mb_ps = minBps.tile([128, 127], F32, tag="minB")
                nc.tensor.matmul(mb_ps[:, :], ones_row128[:, :],
                                 mrow[:, 0:127], start=True, stop=True)
                mbc4 = small.tile([128, 127], F16, tag="mbc4")
                nc.vector.tensor_copy(mbc4[:, :], mb_ps[:, :])

                chains[g] = (mbc4, wts)

            def emit_pv(g):
                i0, G = GROUPS[g]
                GW = G * 128
                eg = egs[g]
                mbc4, wts = chains[g]
                w_sb = wsbs[g]
                halves = [list(range(0, min(2, G))), list(range(2, G))]
                ot_ps = pvps.tile([128, GW], F32, tag="otps")
                for hi, ils in enumerate(halves):
                    if not ils:
                        continue
                    for il in ils:
                        qs0 = il * 128
                        nc.vector.tensor_tensor(
                            eg[:, :, qs0:qs0 + 128],
                            eg[:, :, qs0:qs0 + 128],
                            _bcast_ap(mbc4[:, 32 * il:32 * il + 31], NB, 128,
                                      inner_step=0, mid_step=1),
                            mybir.AluOpType.mult,
                        )
                    c0 = ils[0] * 128
                    c1 = (ils[-1] + 1) * 128
                    for j in range(NB):
                        nc.tensor.matmul(
                            ot_ps[:, c0:c1],
                            sv[:, j, :],
                            eg[:, j, c0:c1],
                            start=(j == 0), stop=(j == NB - 1),
                        )
                for il in range(G):
                    i = i0 + il
                    nv = NVLAST if i == 30 else 128
                    wt_sb, _ = wts[il]
                    dtmp = small.tile([128, 31], F32, tag="dtmp",
                                      name=f"dtmp{g}_{il}")
                    nc.vector.scalar_tensor_tensor(
                        dtmp[:nv, :], wt_sb[:nv, :], 1.0,
                        mbc4[:nv, 32 * il:32 * il + 31],
                        mybir.AluOpType.mult, mybir.AluOpType.mult,
                        accum_out=den_sb[:nv, i:i + 1])
                ot_sb = outsp.tile([128, GW], F32, tag="otsb")
                nc.vector.tensor_copy(ot_sb[:, :], ot_ps[:, :])
                nc.sync.dma_start(outT_d[:, i0 * 128:i0 * 128 + GW],
                                  ot_sb[:, :])
                del egs[g]
                del chains[g]

            # software-pipelined schedule
            emit_st(0, list(range(NB)))
            emit_w(0)
            for g in range(len(GROUPS)):
                nxt = g + 1
                emit_chain_group(g)
                if nxt < len(GROUPS):
                    emit_st(nxt, list(range(NB)))
                    emit_w(nxt)
                emit_pv(g)

            nc.sync.dma_start(den_d[:, :], den_sb[:, :])

    nc.compile()
    return nc


_CACHED = {}


def _get_graph():
    if "nc" not in _CACHED:
        _CACHED["nc"] = build_graph()
    return _CACHED["nc"]


def _prepare_inputs(q, k, v, perm):
    q = np.asarray(q, dtype=np.float32)
    k = np.asarray(k, dtype=np.float32)
    v = np.asarray(v, dtype=np.float32)
    perm = np.asarray(perm, dtype=np.int64)

    def rearr(x):  # [1,8,SEQ,D] -> video permuted first, text appended
        return np.concatenate([x[0, :, TEXT:, :][:, perm, :], x[0, :, :TEXT, :]],
                              axis=1)

    qr, kr, vr = rearr(q), rearr(k), rearr(v)      # [8, SEQ, D]
    i31 = np.eye(31, dtype=np.float32)
    i128 = np.eye(128, dtype=np.float32)
    # basis: bas[r, j, m] = 1 if m == j and key row r of block j is valid
    bas = np.zeros((128, NB, NB), np.float16)
    for j in range(NB):
        kv = NVLAST if j == 30 else 128
        bas[:kv, j, j] = 1.0
    bas = np.ascontiguousarray(bas.reshape(128, NB * NB))
    in_maps = []
    for c in range(NCORES):
        qp = np.zeros((SP, D), np.float16)
        qp[:SEQ] = qr[c]
        kp = np.zeros((SP, D), np.float16)
        kp[:SEQ] = kr[c]
        vp = np.zeros((SP, D), np.float16)
        vp[:SEQ] = vr[c]
        in_maps.append({
            "qT": np.ascontiguousarray(qp.T),
            "kT": np.ascontiguousarray(kp.T),
            "vv": np.ascontiguousarray(
                vp.reshape(NB, 128, D).transpose(1, 0, 2).reshape(128, NB * 128)),
            "bas": bas,
            "i31": i31,
            "i128": i128,
        })
    return in_maps, perm


def run(inputs, trace=False, trace_kwargs=None):
    nc = _get_graph()
    in_maps, perm = _prepare_inputs(inputs["q"], inputs["k"], inputs["v"],
                                    inputs["perm"])
    res = run_bass_kernel_spmd(
        nc, in_maps, core_ids=list(range(NCORES)), trace=trace,
        **(trace_kwargs or {}),
    )
    outs = np.empty((NCORES, SEQ, D), np.float32)
    for c in range(NCORES):
        oT = res.results[c]["outT"][:, :SEQ]          # [D, SEQ]
        den = res.results[c]["den"].T.reshape(SP)[:SEQ]   # den[q%128, i] -> [SEQ]
        outs[c] = (oT / den[None, :]).T
    g2o = np.argsort(perm)
    txt = outs[:, VID:SEQ, :]
    vid = outs[:, :VID, :][:, g2o, :]
    full = np.concatenate([txt, vid], axis=1)[None]   # [1, 8, SEQ, D]
    return np.ascontiguousarray(full.astype(np.float32)), res


def kernel(q, k, v, perm):
    out, _ = run({"q": q, "k": k, "v": v, "perm": perm})
    return out

